# revision 37
# baseline (speedup 1.0000x reference)
"""DisentangledAttention on 8 Trainium2 cores (Bass/Tile).

Problem (hardcoded shapes): x[2,2048,1024], 16 heads x 64 dims, torch-Linear
projections, scores = q@k^T/8 + alpha_h*(pe@pe^T) + beta_h, key-side mask,
softmax, out = (P@v) @ Wo^T + bo.

Sharding: core i = (batch b = i//4, head-group g = i%4, heads 4g..4g+3).
Each core computes its 4 heads' attention and a partial out-projection
[2048,1024] in bf16; host sums the 4 partials per batch (tensor-parallel
unshard).

Math simplifications (exact):
- beta_h is constant along the softmax axis -> cancels. Dropped.
- bk shifts scores by q.bk, constant along key axis -> cancels. Dropped.
- bv contributes sum_k P[q,k] * (bv @ Wo_slice^T) = bv @ Wo^T per row since
  softmax rows sum to 1 -> exact host-side additive correction with bo.
- bq enters scores via bq.k -> per-partition scalar add on q^T (DVE).
- 1/sqrt(64) folded into Wq on host; alpha_h applied on device (DVE scale).
- pos term fused into the QK matmul: q' = [q/8 ; alpha_h*pe], k' = [k ; pe]
  stacked along the contraction dim (64+64=128) -> pos attention is free
  (matmul cost is column-count-bound, not contraction-bound).
- scores are built TRANSPOSED [key, query]: the key-side mask becomes a
  per-partition ACT bias on the exp, and P~^T feeds the PV matmul directly
  (no transpose). Softmax denominators come from a ones-row appended to V
  (M=65 PV matmul); normalization is a reciprocal + GPSIMD partition
  broadcast + multiply, entirely off the PE's critical path.
- no max-subtraction in softmax: scores peak ~9.4 here, exp is f32-safe, and
  softmax is shift-invariant so this matches the reference to rounding.

Engine budget per core per rep (measured on this HW via paired-loop
microbenchmarks: bf16 512-col matmul ~228 ns regardless of lhsT reuse —
LDWEIGHTS pipelines, contrary to the earlier serial-LDW belief; exp
[128,1024] PSUM->SBUF ~1045 ns = ~190 ns fixed + 0.83 ns/col):
  PE  ~176 us  (QK 256 + PV 256 + proj 256 + outproj 64 matmuls)
  ACT ~175 us  (128 exps + 64 out-proj drains)
  DVE ~55 us, GPSIMD ~13 us, DMA ~35 us
PE > ACT-exp, so the kernel is PE-bound: the goal is a PE-saturated
software pipeline. Structure:
- QK runs ONE TILE AHEAD of its exp->PV pair (emitted between exp(t) and
  PV(t), carried across block and rep boundaries), so PV's wait on exp
  never blocks the next QK. Measured on the isolated attention loop:
  1381 -> 1322 ns/tile (a 3rd score slot would give 1224, but PSUM's 8
  banks only fit score ping/pong + the PV accumulator + 2 proj banks).
- ALL other PE work is injected into the attention stream as fine-grained
  generator steps (<=2 matmuls each, one step per tile): rep r-1's
  chunk-1 out-projection (blocks 0-3), rep r's chunk-0 out-projection
  (blocks 4-7), and ALL of rep r+1's q/k/v projections + DMAs + pos-prep
  spread across the rep. The PE rolls from rep r's last PV directly into
  rep r+1's first QK with no serial projection phase. One step per tile
  is the measured optimum - forcing 2 burns the exp stream.
- out-projection drains (PSUM->SBUF bf16) run on ACT (scalar.copy), NOT
  DVE, whose q/k/v drains + normalize chain they'd contend with. GPSIMD
  cannot read PSUM; DMA cannot read PSUM; output DMA'd as bf16 partials.
- Tiles live in flat pools with per-rep TAGS (bufs=2 rings + WAR sems)
  so emission order is free of the pool-stack LIFO constraint. Only rep
  0's projections are serial per For_i body, amortized over the reps.
Dead ends measured on this problem: fp8e4m3 anywhere in the data path
(PV DoubleRow, projections, out-projection) fails the 2e-2 gate because
the softmax rows are quasi-one-hot (scores peak 9.4), so per-element fp8
noise (~4-6%) passes undamped to the output (measured 3.7e-2..4.1e-2);
half-width (512) exps lose to ACT per-instruction overhead; real data
runs the PE ~6% slower than zeros (power), so zero-input microbenches
underestimate. Steady state: 258 -> 228 us measured (12-rep unroll).
"""

import numpy as np

import concourse.bacc as bacc
import concourse.bass as bass
import concourse.mybir as mybir
import concourse.tile as tile
from concourse.bass import ds, ts
from concourse.bass_utils import run_bass_kernel_spmd

B = 2
S = 2048
D = 1024
H = 16
DH = 64
NCORES = 8
GROUPS = 4          # head-groups
HPC = H // GROUPS   # heads per core = 4
O = HPC * DH        # out dims per core = 256
KD = D // 128       # contraction tiles over d_model = 8
ST = S // 128       # seq tiles of 128 = 16
SC = S // 512       # seq chunks of 512 = 4
CW = 1024           # exp/score chunk width
NCH = S // CW       # chunks per rep = 2

F32 = mybir.dt.float32
BF16 = mybir.dt.bfloat16
BF16_NP = mybir.dt.np(BF16)
_CACHE = {}


class _null_ctx:
    def __enter__(self):
        return None

    def __exit__(self, *a):
        return False


def _build(reps: int = 1, loop_n: int | None = None, ablate: str = ""):
    """Build the kernel program.

    reps: unrolled copies of the body (bench uses >1 to amortize).
    loop_n: if set, wrap the reps in a For_i hardware loop of this count
    (bench-only — lets timing loops run long enough to dominate the
    ~30-90ms tunnel dispatch jitter without exploding program size).
    """
    nc = bacc.Bacc("TRN2", target_bir_lowering=False, debug=False, num_devices=NCORES)

    # x / weights arrive pre-laid-out by the host in exactly the SBUF tile
    # shape ([partition, k-tile, free]), so each loads with ONE fully
    # contiguous DMA
    xT = nc.dram_tensor("xT", [128, KD, S], BF16, kind="ExternalInput").ap()
    wqT = nc.dram_tensor("wqT", [128, KD, O], BF16, kind="ExternalInput").ap()
    wkT = nc.dram_tensor("wkT", [128, KD, O], BF16, kind="ExternalInput").ap()
    wvT = nc.dram_tensor("wvT", [128, KD, O], BF16, kind="ExternalInput").ap()
    woT = nc.dram_tensor("woT", [128, 2, D], BF16, kind="ExternalInput").ap()
    peT = nc.dram_tensor("peT", [DH, S], BF16, kind="ExternalInput").ap()
    # smalls: [:, 0:2] = bq/8 by head-pair, [:, 2:18] = mask bias by key
    # tile, [0:64, 18:22] = per-head alpha replicated down 64 partitions
    smalls_d = nc.dram_tensor(
        "smalls", [128, 2 + ST + HPC], F32, kind="ExternalInput"
    ).ap()
    out_d = nc.dram_tensor("out", [S, D], BF16, kind="ExternalOutput").ap()

    with tile.TileContext(nc) as tc:
     with tc.For_i(0, loop_n, 1) if loop_n else _null_ctx() as _i:
      with tc.tile_pool(name="sb", bufs=1) as sb, \
           tc.tile_pool(name="ps", bufs=1, space="PSUM") as ps:
        state: dict[int, dict] = {}

        def alloc_rep(r):
            t = {}
            t["xT"] = sb.tile([128, KD, S], BF16, tag="xT", bufs=1,
                              name=f"xT_{r}")
            t["wq"] = sb.tile([128, KD, O], BF16, tag="wq", bufs=1,
                              name=f"wq_{r}")
            t["wk"] = sb.tile([128, KD, O], BF16, tag="wk", bufs=1,
                              name=f"wk_{r}")
            t["wv"] = sb.tile([128, KD, O], BF16, tag="wv", bufs=1,
                              name=f"wv_{r}")
            t["smalls"] = sb.tile([128, 2 + ST + HPC], F32, tag="smalls",
                                  bufs=2, name=f"smalls_{r}")
            t["peT"] = sb.tile([DH, S], BF16, tag="peT", bufs=2,
                               name=f"peT_{r}")
            t["woT"] = sb.tile([128, 2, D], BF16, tag="woT", bufs=2,
                               name=f"woT_{r}")
            t["qp"] = [sb.tile([128, S], BF16, tag=f"qp{h}", bufs=2,
                               name=f"qp{h}_{r}") for h in range(HPC)]
            t["kp"] = [sb.tile([128, S], BF16, tag=f"kp{h}", bufs=2,
                               name=f"kp{h}_{r}") for h in range(HPC)]
            t["vp"] = sb.tile([128, ST, HPC, DH + 1], BF16, tag="vp", bufs=2,
                              name=f"vp_{r}")
            t["attnT"] = [sb.tile([128, S], BF16, tag=f"attnT{k}", bufs=2,
                                  name=f"attnT{k}_{r}") for k in range(2)]
            return t

        def gen_dma_in(t):
            # DMA order = need order: x/weights gate the first proj matmuls;
            # pos-embed and Wo are consumed much later
            nc.sync.dma_start(out=t["xT"], in_=xT)
            yield
            nc.sync.dma_start(out=t["wq"], in_=wqT)
            nc.sync.dma_start(out=t["wk"], in_=wkT)
            nc.sync.dma_start(out=t["wv"], in_=wvT)
            yield
            nc.sync.dma_start(out=t["smalls"], in_=smalls_d)
            nc.sync.dma_start(out=t["peT"], in_=peT)
            nc.sync.dma_start(out=t["woT"], in_=woT)

        def gen_prep(t):
            # pos halves: even head of a pair keeps content in rows 0:64 /
            # pos in 64:128, odd head the reverse (both sides of the QK
            # matmul use the same order, so dots match). q side is scaled by
            # the head's alpha (data-driven). DVE, not ACT: the ACT queue is
            # reserved for the exps that pace the attention stream.
            nc.vector.memset(t["vp"][:, :, :, DH : DH + 1], 1.0)
            alphas = t["smalls"][0:64, 2 + ST : 2 + ST + HPC]
            for h in range(HPC):
                crow = (h % 2) * 64
                prow = 64 - crow
                nc.vector.tensor_scalar_mul(
                    t["qp"][h][prow : prow + 64, :],
                    t["peT"],
                    alphas[:, h : h + 1],
                )
                yield
                nc.vector.tensor_copy(
                    out=t["kp"][h][prow : prow + 64, :],
                    in_=t["peT"],
                )
                if h < HPC - 1:
                    yield

        def gen_q_group(t, hp, c):
            q_ps = ps.tile([128, 512], F32, tag="qk_ps", bufs=2,
                           name=f"q_ps_{hp}_{c}")
            for kd in range(KD):
                nc.tensor.matmul(
                    out=q_ps,
                    lhsT=t["wq"][:, kd, ts(hp, 128)],
                    rhs=t["xT"][:, kd, ds(c * 512, 512)],
                    start=(kd == 0),
                    stop=(kd == KD - 1),
                )
                if kd % 2 == 1:
                    yield
            bqsb = t["smalls"][:, 0:2]
            for par in range(2):  # even/odd head of pair
                h = 2 * hp + par
                crow = (h % 2) * 64
                nc.vector.tensor_scalar_add(
                    t["qp"][h][crow : crow + 64, ds(c * 512, 512)],
                    q_ps[crow : crow + 64, :],
                    bqsb[crow : crow + 64, hp : hp + 1],
                )

        def gen_k_group(t, hp, c):
            k_ps = ps.tile([128, 512], F32, tag="qk_ps", bufs=2,
                           name=f"k_ps_{hp}_{c}")
            for kd in range(KD):
                nc.tensor.matmul(
                    out=k_ps,
                    lhsT=t["wk"][:, kd, ts(hp, 128)],
                    rhs=t["xT"][:, kd, ds(c * 512, 512)],
                    start=(kd == 0),
                    stop=(kd == KD - 1),
                )
                if kd % 2 == 1:
                    yield
            for par in range(2):
                h = 2 * hp + par
                crow = (h % 2) * 64
                nc.vector.tensor_copy(
                    out=t["kp"][h][crow : crow + 64, ds(c * 512, 512)],
                    in_=k_ps[crow : crow + 64, :],
                )

        def gen_v_group(t, st):
            v_ps = ps.tile([128, O], F32, tag="qk_ps", bufs=2,
                           name=f"v_ps_{st}")
            for kd in range(KD):
                nc.tensor.matmul(
                    out=v_ps,
                    lhsT=t["xT"][:, kd, ts(st, 128)],
                    rhs=t["wv"][:, kd, :],
                    start=(kd == 0),
                    stop=(kd == KD - 1),
                )
                if kd % 2 == 1:
                    yield
            nc.vector.tensor_copy(
                out=t["vp"][:, st, :, 0:DH],
                in_=v_ps.rearrange("p (h d) -> p h d", h=HPC),
            )

        def gen_outproj_unit(t, st0):
            # partial out-projection for seq tiles st0, st0+1 (host sums
            # over head-groups). Two s-tiles share one bf16 staging tile
            # and one DMA. Drains run on ACT (~84us idle under the
            # PE-bound wall), not DVE, whose drains+normalize chain they
            # would otherwise contend with.
            o_sb2 = sb.tile([128, 2, D], BF16, tag="osb2", bufs=2,
                            name=f"osb2_{st0}")
            for j in range(2):
                st = st0 + j
                for nk in range(2):
                    o_ps = ps.tile([128, 512], F32, tag="qk_ps", bufs=2,
                                   name=f"o_ps_{st}_{nk}")
                    for kt in range(2):
                        nc.tensor.matmul(
                            out=o_ps,
                            lhsT=t["attnT"][kt][:, ts(st, 128)],
                            rhs=t["woT"][:, kt, ds(nk * 512, 512)],
                            start=(kt == 0),
                            stop=(kt == 1),
                        )
                    nc.scalar.copy(
                        out=o_sb2[:, j, ds(nk * 512, 512)], in_=o_ps
                    )
                    yield
            nc.sync.dma_start(
                out=out_d[ds(st0 * 128, 256), :].rearrange(
                    "(two p) d -> p two d", p=128
                ),
                in_=o_sb2,
            )

        def proj_gens(t):
            # Order: hp0's k (all chunks) and q (chunk-0 cols) first so the
            # next rep's block (0,0) unblocks earliest; v st asc for its
            # JIT reads; hp1 and high chunks later.
            seq = [
                ("k", 0, 0), ("k", 0, 1), ("q", 0, 0), ("v", 0),
                ("k", 0, 2), ("q", 0, 1), ("v", 1), ("k", 0, 3),
                ("v", 2), ("v", 3), ("q", 0, 2), ("v", 4),
                ("q", 0, 3), ("v", 5), ("k", 1, 0), ("v", 6),
                ("k", 1, 1), ("v", 7), ("q", 1, 0), ("v", 8),
                ("k", 1, 2), ("v", 9), ("q", 1, 1), ("v", 10),
                ("k", 1, 3), ("v", 11), ("q", 1, 2), ("v", 12),
                ("q", 1, 3), ("v", 13), ("v", 14), ("v", 15),
            ]
            gens = []
            for item in seq:
                if item[0] == "q":
                    gens.append(gen_q_group(t, item[1], item[2]))
                elif item[0] == "k":
                    gens.append(gen_k_group(t, item[1], item[2]))
                else:
                    gens.append(gen_v_group(t, item[1]))
            return gens

        def run_gens(gens):
            for g in gens:
                for _ in g:
                    pass

        class Stepper:
            """Round-robin advance a list of generators, one step at a time."""

            def __init__(self, gens):
                self.gens = list(gens)
                self.i = 0

            def step(self) -> bool:
                while self.i < len(self.gens):
                    try:
                        next(self.gens[self.i])
                        return True
                    except StopIteration:
                        self.i += 1
                return False

            def drain(self):
                while self.step():
                    pass

        def emit_qk(r, c, h, tt):
            """QK for one tile -> fresh s_ps; returned for the matching exp."""
            t = state[r]
            s_ps = ps.tile([128, CW], F32, tag="u", bufs=3,
                           name=f"s_ps_{r}_{c}_{h}_{tt}")
            for half in range(CW // 512):
                nc.tensor.matmul(
                    out=s_ps[:, ds(half * 512, 512)],
                    lhsT=t["kp"][h][:, ts(tt, 128)],
                    rhs=t["qp"][h][:, ds(c * CW + half * 512, 512)],
                    start=True,
                    stop=True,
                )
            return s_ps

        carry = {}  # s_ps carried into the next rep's first tile

        def emit_attention(r, main_q: Stepper, late_q: Stepper,
                           carry_out: bool):
            """8 blocks x 16 tiles, QK software-pipelined one tile ahead of
            the exp->PV pair so PV's wait on exp never blocks the next QK.
            main_q steps spread over all 128 tiles, late_q steps over blocks
            4-7 (needs chunk-0 attnT complete). With carry_out, the last
            lookahead QK is the NEXT rep's first tile (its q/k were projected
            during this rep), so the PE rolls across the rep boundary."""
            t = state[r]
            maskb = t["smalls"][:, 2 : 2 + ST]
            tiles = [(c, h, tt) for c in range(NCH) for h in range(HPC)
                     for tt in range(ST)]
            s_cur = carry.pop(r, None)
            if s_cur is None:
                s_cur = emit_qk(r, *tiles[0])
            z_ps = None
            for i, (c, h, tt) in enumerate(tiles):
                bi = i // ST
                if tt == 0:
                    z_ps = ps.tile([DH + 1, CW], F32, tag="u", bufs=3,
                                   name=f"z_ps_{r}_{c}_{h}")
                main_q.step()
                if bi >= 4:
                    late_q.step()
                p_sb = sb.tile([128, CW], BF16, tag="p", bufs=2,
                               name=f"p_{c}_{h}_{tt}")
                nc.scalar.activation(
                    out=p_sb,
                    in_=s_cur,
                    func=mybir.ActivationFunctionType.Exp,
                    bias=maskb[:, tt : tt + 1],
                    scale=1.0,
                )
                if i + 1 < len(tiles):
                    s_cur = emit_qk(r, *tiles[i + 1])
                elif carry_out:
                    carry[r + 1] = emit_qk(r + 1, *tiles[0])
                for half in range(CW // 512):
                    nc.tensor.matmul(
                        out=z_ps[:, ds(half * 512, 512)],
                        lhsT=t["vp"][:, tt, h, :],
                        rhs=p_sb[:, ds(half * 512, 512)],
                        start=(tt == 0),
                        stop=(tt == ST - 1),
                    )
                if tt == ST - 1 and "nonorm" not in ablate:
                    # normalize: 1/denominator broadcast down 64 partitions
                    # on GPSIMD keeps the whole chain off the PE and ACT
                    z_sb = sb.tile([DH + 1, CW], F32, tag="zsb", bufs=2,
                                   name=f"z_sb_{c}_{h}")
                    nc.vector.tensor_copy(out=z_sb, in_=z_ps)
                    recip = sb.tile([1, CW], F32, tag="recip", bufs=2,
                                    name=f"recip_{c}_{h}")
                    nc.vector.reciprocal(recip, z_sb[DH : DH + 1, :])
                    bc_sb = sb.tile([64, CW], F32, tag="bc_sb", bufs=2,
                                    name=f"bc_{c}_{h}")
                    nc.gpsimd.partition_broadcast(bc_sb, recip)
                    row = (h % 2) * 64
                    nc.vector.tensor_mul(
                        out=t["attnT"][h // 2][row : row + 64,
                                               ds(c * CW, CW)],
                        in0=z_sb[0:DH, :],
                        in1=bc_sb,
                    )
            main_q.drain()
            late_q.drain()

        # ---- body: software pipeline over reps ----
        state[0] = alloc_rep(0)
        run_gens([gen_dma_in(state[0]), gen_prep(state[0])])
        run_gens(proj_gens(state[0]))

        for r in range(reps):
            main = []
            if r >= 1 and "nooutproj" not in ablate:
                # previous rep's chunk-1 out-projection
                for st0 in range(8, ST, 2):
                    main.append(gen_outproj_unit(state[r - 1], st0))
            if r + 1 < reps:
                state[r + 1] = alloc_rep(r + 1)
                main.append(gen_dma_in(state[r + 1]))
                main.append(gen_prep(state[r + 1]))
                if "noproj" not in ablate:
                    main.extend(proj_gens(state[r + 1]))
            late = ([] if "nooutproj" in ablate else
                    [gen_outproj_unit(state[r], st0) for st0 in range(0, 8, 2)])
            emit_attention(r, Stepper(main), Stepper(late),
                           carry_out=(r + 1 < reps))

        # tail: last rep's chunk-1 out-projection
        if "nooutproj" not in ablate:
            run_gens([gen_outproj_unit(state[reps - 1], st0)
                      for st0 in range(8, ST, 2)])

    nc.compile()
    return nc


def kernel(
    x, mask, Wq, bq, Wk, bk, Wv, bv, Wo, bo, pos_embed, alpha, beta, **_unused
):
    x = np.asarray(x, dtype=np.float32)
    mask = np.asarray(mask)
    Wq = np.asarray(Wq, dtype=np.float32)
    Wk = np.asarray(Wk, dtype=np.float32)
    Wv = np.asarray(Wv, dtype=np.float32)
    Wo = np.asarray(Wo, dtype=np.float32)
    bq = np.asarray(bq, dtype=np.float32)
    bv = np.asarray(bv, dtype=np.float32)
    bo = np.asarray(bo, dtype=np.float32)
    pe = np.asarray(pos_embed, dtype=np.float32)
    alpha = np.asarray(alpha, dtype=np.float32).reshape(H)

    if "nc" not in _CACHE:
        _CACHE["nc"] = _build()
    nc = _CACHE["nc"]

    scale = np.float32(1.0 / np.sqrt(DH))
    peT_np = np.ascontiguousarray(pe.T)
    maskbias = np.where(mask == 0, np.float32(-1e9), np.float32(0.0)).astype(
        np.float32
    )

    in_maps = []
    for core in range(NCORES):
        b, g = divmod(core, GROUPS)
        osl = slice(g * O, (g + 1) * O)
        heads = list(range(g * HPC, (g + 1) * HPC))
        smalls = np.zeros((128, 2 + ST + HPC), np.float32)
        smalls[:, 0:2] = (bq[osl] * scale).reshape(2, 128).T
        smalls[:, 2 : 2 + ST] = maskbias[b].reshape(ST, 128).T
        smalls[0:64, 2 + ST :] = alpha[heads][None, :]

        def sb_layout(mat_T, kt):
            # [rows, cols] -> [128, kt, cols]: row r = k*128 + p -> [p][k]
            r, cols = mat_T.shape
            return np.ascontiguousarray(
                mat_T.reshape(kt, 128, cols).transpose(1, 0, 2)
            )

        in_maps.append(
            {
                "xT": sb_layout(x[b].T, KD).astype(BF16_NP),
                "wqT": sb_layout((Wq[osl] * scale).T, KD).astype(BF16_NP),
                "wkT": sb_layout(Wk[osl].T, KD).astype(BF16_NP),
                "wvT": sb_layout(Wv[osl].T, KD).astype(BF16_NP),
                "woT": sb_layout(Wo[:, osl].T, 2).astype(BF16_NP),
                "peT": peT_np.astype(BF16_NP),
                "smalls": smalls,
                "out": np.zeros((S, D), BF16_NP),
            }
        )

    _CACHE["in_maps"] = in_maps
    # the axon-tunneled devices intermittently fault (NRT_EXEC_UNIT_
    # UNRECOVERABLE); a retry on a fresh call recovers
    for attempt in range(3):
        try:
            res = run_bass_kernel_spmd(nc, in_maps, core_ids=list(range(NCORES)))
            break
        except Exception:
            if attempt == 2:
                raise

    correction = Wo @ bv + bo  # exact bv/bo contribution (see module docstring)
    out = np.empty((B, S, D), np.float32)
    for b in range(B):
        acc = np.zeros((S, D), np.float64)
        for g in range(GROUPS):
            acc += res.results[b * GROUPS + g]["out"].astype(np.float32)
        out[b] = (acc + correction).astype(np.float32)
    return out


# revision 39
# speedup vs baseline: 1.1327x; 1.1327x over previous
"""DisentangledAttention on 8 Trainium2 cores (Bass/Tile).

Problem (hardcoded shapes): x[2,2048,1024], 16 heads x 64 dims, torch-Linear
projections, scores = q@k^T/8 + alpha_h*(pe@pe^T) + beta_h, key-side mask,
softmax, out = (P@v) @ Wo^T + bo.

Sharding: core i = (batch b = i//4, head-group g = i%4, heads 4g..4g+3).
Each core computes its 4 heads' attention and a partial out-projection
[2048,1024] in bf16; host sums the 4 partials per batch (tensor-parallel
unshard).

Math simplifications (exact):
- beta_h is constant along the softmax axis -> cancels. Dropped.
- bk shifts scores by q.bk, constant along key axis -> cancels. Dropped.
- bv contributes sum_k P[q,k] * (bv @ Wo_slice^T) = bv @ Wo^T per row since
  softmax rows sum to 1 -> exact host-side additive correction with bo.
- bq enters scores via bq.k -> per-partition scalar add on q^T (DVE).
- 1/sqrt(64) folded into Wq on host; alpha_h applied on device (DVE scale).
- pos term fused into the QK matmul: q' = [q/8 ; alpha_h*pe], k' = [k ; pe]
  stacked along the contraction dim (64+64=128) -> pos attention is free
  (matmul cost is column-count-bound, not contraction-bound).
- scores are built TRANSPOSED [key, query]: the key-side mask becomes a
  per-partition ACT bias on the exp, and P~^T feeds the PV matmul directly
  (no transpose). Softmax denominators come from a ones-row appended to V
  (M=65 PV matmul); normalization is a reciprocal + GPSIMD partition
  broadcast + multiply, entirely off the PE's critical path.
- no max-subtraction in softmax: scores peak ~9.4 here, exp is f32-safe, and
  softmax is shift-invariant so this matches the reference to rounding.

Engine budget per core per rep (measured on this HW via paired-loop
microbenchmarks: bf16 512-col matmul ~228 ns regardless of lhsT reuse —
LDWEIGHTS pipelines; exp [128,1024] PSUM->SBUF ~1045 ns):
  PE  ~176 us  (QK 256 + PV 256 + proj 256 + outproj 64 matmuls)
  ACT ~134 us  (128 exps)
  DVE ~81 us, GPSIMD ~13 us, DMA ~35 us
PE > ACT, so the kernel is PE-bound: the goal is a PE-saturated software
pipeline. Attention tile t costs the PE 912 ns (2 QK + 2 PV) under a
1045 ns exp, leaving ~130 ns/tile of PE slack plus whatever the stream
stretches: ALL other PE work is injected into the attention stream as
fine-grained steps (<=2 matmuls each, one step per tile):
  - rep r-1's chunk-1 out-projection  -> rep r blocks 0-3
  - rep r's own chunk-0 out-projection -> rep r blocks 4-7
  - ALL of rep r+1's q/k/v projections (+ their DMAs and pos-embed prep)
    -> spread over rep r's whole attention stream
With that, the PE rolls from rep r's last PV directly into rep r+1's
first QK with no projection phase in between; ACT runs ~76% busy and the
steady state approaches the PE roofline. Tiles live in flat pools with
per-rep tags (bufs=2 rings + WAR sems) so emission order is free of the
pool-stack LIFO constraint; the Tile scheduler (time-aware list
scheduler) smooths the rest. Only rep 0's projections (+first DMA) are
serial per For_i body, amortized over the unrolled reps.
"""

import numpy as np

import concourse.bacc as bacc
import concourse.bass as bass
import concourse.mybir as mybir
import concourse.tile as tile
from concourse.bass import ds, ts
from concourse.bass_utils import run_bass_kernel_spmd

B = 2
S = 2048
D = 1024
H = 16
DH = 64
NCORES = 8
GROUPS = 4          # head-groups
HPC = H // GROUPS   # heads per core = 4
O = HPC * DH        # out dims per core = 256
KD = D // 128       # contraction tiles over d_model = 8
ST = S // 128       # seq tiles of 128 = 16
SC = S // 512       # seq chunks of 512 = 4
CW = 1024           # exp/score chunk width
NCH = S // CW       # chunks per rep = 2

F32 = mybir.dt.float32
BF16 = mybir.dt.bfloat16
BF16_NP = mybir.dt.np(BF16)
FP8 = mybir.dt.float8e4
FP8_NP = mybir.dt.np(FP8)

_CACHE = {}


class _null_ctx:
    def __enter__(self):
        return None

    def __exit__(self, *a):
        return False


def _build(reps: int = 1, loop_n: int | None = None, ablate: str = ""):
    """Build the kernel program.

    reps: unrolled copies of the body (bench uses >1 to amortize).
    loop_n: if set, wrap the reps in a For_i hardware loop of this count
    (bench-only — lets timing loops run long enough to dominate the
    ~30-90ms tunnel dispatch jitter without exploding program size).
    """
    nc = bacc.Bacc("TRN2", target_bir_lowering=False, debug=False, num_devices=NCORES)

    # x / weights arrive pre-laid-out by the host in exactly the SBUF tile
    # shape ([partition, k-tile, free]), so each loads with ONE fully
    # contiguous DMA
    xT = nc.dram_tensor("xT", [128, KD, S], BF16, kind="ExternalInput").ap()
    wqT = nc.dram_tensor("wqT", [128, KD, O], BF16, kind="ExternalInput").ap()
    wkT = nc.dram_tensor("wkT", [128, KD, O], BF16, kind="ExternalInput").ap()
    wvT = nc.dram_tensor("wvT", [128, KD, O], BF16, kind="ExternalInput").ap()
    woT = nc.dram_tensor("woT", [128, 2, D], BF16, kind="ExternalInput").ap()
    peT = nc.dram_tensor("peT", [DH, S], BF16, kind="ExternalInput").ap()
    # smalls: [:, 0:2] = bq/8 by head-pair, [:, 2:18] = mask bias by key
    # tile, [0:64, 18:22] = per-head alpha replicated down 64 partitions
    smalls_d = nc.dram_tensor(
        "smalls", [128, 2 + ST + HPC], F32, kind="ExternalInput"
    ).ap()
    out_d = nc.dram_tensor("out", [S, D], BF16, kind="ExternalOutput").ap()

    with tile.TileContext(nc) as tc:
     with tc.For_i(0, loop_n, 1) if loop_n else _null_ctx() as _i:
      with tc.tile_pool(name="sb", bufs=1) as sb, \
           tc.tile_pool(name="ps", bufs=1, space="PSUM") as ps:
        state: dict[int, dict] = {}

        def alloc_rep(r):
            t = {}
            t["xT"] = sb.tile([128, KD, S], BF16, tag="xT", bufs=1,
                              name=f"xT_{r}")
            t["wq"] = sb.tile([128, KD, O], BF16, tag="wq", bufs=1,
                              name=f"wq_{r}")
            t["wk"] = sb.tile([128, KD, O], BF16, tag="wk", bufs=1,
                              name=f"wk_{r}")
            t["wv"] = sb.tile([128, KD, O], BF16, tag="wv", bufs=1,
                              name=f"wv_{r}")
            t["smalls"] = sb.tile([128, 2 + ST + HPC], F32, tag="smalls",
                                  bufs=2, name=f"smalls_{r}")
            t["peT"] = sb.tile([DH, S], BF16, tag="peT", bufs=2,
                               name=f"peT_{r}")
            t["woT"] = sb.tile([128, 2, D], BF16, tag="woT", bufs=2,
                               name=f"woT_{r}")
            t["qp"] = [sb.tile([128, S], BF16, tag=f"qp{h}", bufs=2,
                               name=f"qp{h}_{r}") for h in range(HPC)]
            t["kp"] = [sb.tile([128, S], BF16, tag=f"kp{h}", bufs=2,
                               name=f"kp{h}_{r}") for h in range(HPC)]
            t["vp"] = sb.tile([128, ST, HPC, DH + 1], BF16, tag="vp", bufs=2,
                              name=f"vp_{r}")
            t["attnT"] = [sb.tile([128, S], BF16, tag=f"attnT{k}", bufs=2,
                                  name=f"attnT{k}_{r}") for k in range(2)]
            return t

        def gen_dma_in(t):
            # DMA order = need order: x/weights gate the first proj matmuls;
            # pos-embed and Wo are consumed much later
            nc.sync.dma_start(out=t["xT"], in_=xT)
            yield
            nc.sync.dma_start(out=t["wq"], in_=wqT)
            nc.sync.dma_start(out=t["wk"], in_=wkT)
            nc.sync.dma_start(out=t["wv"], in_=wvT)
            yield
            nc.sync.dma_start(out=t["smalls"], in_=smalls_d)
            nc.sync.dma_start(out=t["peT"], in_=peT)
            nc.sync.dma_start(out=t["woT"], in_=woT)

        def gen_prep(t):
            # pos halves: even head of a pair keeps content in rows 0:64 /
            # pos in 64:128, odd head the reverse (both sides of the QK
            # matmul use the same order, so dots match). q side is scaled by
            # the head's alpha (data-driven). DVE, not ACT: the ACT queue is
            # reserved for the exps that pace the attention stream.
            nc.vector.memset(t["vp"][:, :, :, DH : DH + 1], 1.0)
            alphas = t["smalls"][0:64, 2 + ST : 2 + ST + HPC]
            for h in range(HPC):
                crow = (h % 2) * 64
                prow = 64 - crow
                nc.vector.tensor_scalar_mul(
                    t["qp"][h][prow : prow + 64, :],
                    t["peT"],
                    alphas[:, h : h + 1],
                )
                yield
                nc.vector.tensor_copy(
                    out=t["kp"][h][prow : prow + 64, :],
                    in_=t["peT"],
                )
                if h < HPC - 1:
                    yield

        def gen_q_group(t, hp, c):
            q_ps = ps.tile([128, 512], F32, tag="qk_ps", bufs=2,
                           name=f"q_ps_{hp}_{c}")
            for kd in range(KD):
                nc.tensor.matmul(
                    out=q_ps,
                    lhsT=t["wq"][:, kd, ts(hp, 128)],
                    rhs=t["xT"][:, kd, ds(c * 512, 512)],
                    start=(kd == 0),
                    stop=(kd == KD - 1),
                )
                if kd % 2 == 1:
                    yield
            bqsb = t["smalls"][:, 0:2]
            for par in range(2):  # even/odd head of pair
                h = 2 * hp + par
                crow = (h % 2) * 64
                nc.vector.tensor_scalar_add(
                    t["qp"][h][crow : crow + 64, ds(c * 512, 512)],
                    q_ps[crow : crow + 64, :],
                    bqsb[crow : crow + 64, hp : hp + 1],
                )

        def gen_k_group(t, hp, c):
            k_ps = ps.tile([128, 512], F32, tag="qk_ps", bufs=2,
                           name=f"k_ps_{hp}_{c}")
            for kd in range(KD):
                nc.tensor.matmul(
                    out=k_ps,
                    lhsT=t["wk"][:, kd, ts(hp, 128)],
                    rhs=t["xT"][:, kd, ds(c * 512, 512)],
                    start=(kd == 0),
                    stop=(kd == KD - 1),
                )
                if kd % 2 == 1:
                    yield
            for par in range(2):
                h = 2 * hp + par
                crow = (h % 2) * 64
                nc.vector.tensor_copy(
                    out=t["kp"][h][crow : crow + 64, ds(c * 512, 512)],
                    in_=k_ps[crow : crow + 64, :],
                )

        def gen_v_group(t, st):
            v_ps = ps.tile([128, O], F32, tag="qk_ps", bufs=2,
                           name=f"v_ps_{st}")
            for kd in range(KD):
                nc.tensor.matmul(
                    out=v_ps,
                    lhsT=t["xT"][:, kd, ts(st, 128)],
                    rhs=t["wv"][:, kd, :],
                    start=(kd == 0),
                    stop=(kd == KD - 1),
                )
                if kd % 2 == 1:
                    yield
            nc.vector.tensor_copy(
                out=t["vp"][:, st, :, 0:DH],
                in_=v_ps.rearrange("p (h d) -> p h d", h=HPC),
            )

        def gen_outproj_unit(t, st0):
            # partial out-projection for seq tiles st0, st0+1 (host sums
            # over head-groups). Two s-tiles share one bf16 staging tile
            # and one DMA. Drains run on ACT (~84us idle under the
            # PE-bound wall), not DVE, whose drains+normalize chain they
            # would otherwise contend with.
            o_sb2 = sb.tile([128, 2, D], BF16, tag="osb2", bufs=2,
                            name=f"osb2_{st0}")
            for j in range(2):
                st = st0 + j
                for nk in range(2):
                    o_ps = ps.tile([128, 512], F32, tag="qk_ps", bufs=2,
                                   name=f"o_ps_{st}_{nk}")
                    for kt in range(2):
                        nc.tensor.matmul(
                            out=o_ps,
                            lhsT=t["attnT"][kt][:, ts(st, 128)],
                            rhs=t["woT"][:, kt, ds(nk * 512, 512)],
                            start=(kt == 0),
                            stop=(kt == 1),
                        )
                    nc.scalar.copy(
                        out=o_sb2[:, j, ds(nk * 512, 512)], in_=o_ps
                    )
                    yield
            nc.sync.dma_start(
                out=out_d[ds(st0 * 128, 256), :].rearrange(
                    "(two p) d -> p two d", p=128
                ),
                in_=o_sb2,
            )

        def proj_gens(t):
            # Order: hp0's k (all chunks) and q (chunk-0 cols) first so the
            # next rep's block (0,0) unblocks earliest; v st asc for its
            # JIT reads; hp1 and high chunks later.
            seq = [
                ("k", 0, 0), ("k", 0, 1), ("q", 0, 0), ("v", 0),
                ("k", 0, 2), ("q", 0, 1), ("v", 1), ("k", 0, 3),
                ("v", 2), ("v", 3), ("q", 0, 2), ("v", 4),
                ("q", 0, 3), ("v", 5), ("k", 1, 0), ("v", 6),
                ("k", 1, 1), ("v", 7), ("q", 1, 0), ("v", 8),
                ("k", 1, 2), ("v", 9), ("q", 1, 1), ("v", 10),
                ("k", 1, 3), ("v", 11), ("q", 1, 2), ("v", 12),
                ("q", 1, 3), ("v", 13), ("v", 14), ("v", 15),
            ]
            gens = []
            for item in seq:
                if item[0] == "q":
                    gens.append(gen_q_group(t, item[1], item[2]))
                elif item[0] == "k":
                    gens.append(gen_k_group(t, item[1], item[2]))
                else:
                    gens.append(gen_v_group(t, item[1]))
            return gens

        def run_gens(gens):
            for g in gens:
                for _ in g:
                    pass

        class Stepper:
            """Round-robin advance a list of generators, one step at a time."""

            def __init__(self, gens):
                self.gens = list(gens)
                self.i = 0

            def step(self) -> bool:
                while self.i < len(self.gens):
                    try:
                        next(self.gens[self.i])
                        return True
                    except StopIteration:
                        self.i += 1
                return False

            def drain(self):
                while self.step():
                    pass

        def emit_qk(r, c, h, tt):
            """QK for one tile -> fresh s_ps; returned for the matching exp."""
            t = state[r]
            s_ps = ps.tile([128, CW], F32, tag="u", bufs=3,
                           name=f"s_ps_{r}_{c}_{h}_{tt}")
            for half in range(CW // 512):
                nc.tensor.matmul(
                    out=s_ps[:, ds(half * 512, 512)],
                    lhsT=t["kp"][h][:, ts(tt, 128)],
                    rhs=t["qp"][h][:, ds(c * CW + half * 512, 512)],
                    start=True,
                    stop=True,
                )
            return s_ps

        carry = {}  # s_ps carried into the next rep's first tile

        def emit_attention(r, main_q: Stepper, late_q: Stepper,
                           carry_out: bool):
            """8 blocks x 16 tiles, QK software-pipelined one tile ahead of
            the exp->PV pair so PV's wait on exp never blocks the next QK.
            main_q steps spread over all 128 tiles, late_q steps over blocks
            4-7 (needs chunk-0 attnT complete). With carry_out, the last
            lookahead QK is the NEXT rep's first tile (its q/k were projected
            during this rep), so the PE rolls across the rep boundary."""
            t = state[r]
            maskb = t["smalls"][:, 2 : 2 + ST]
            tiles = [(c, h, tt) for c in range(NCH) for h in range(HPC)
                     for tt in range(ST)]
            s_cur = carry.pop(r, None)
            if s_cur is None:
                s_cur = emit_qk(r, *tiles[0])
            z_ps = None
            for i, (c, h, tt) in enumerate(tiles):
                bi = i // ST
                if tt == 0:
                    z_ps = ps.tile([DH + 1, CW], F32, tag="u", bufs=3,
                                   name=f"z_ps_{r}_{c}_{h}")
                main_q.step()
                if bi >= 4:
                    late_q.step()
                p_sb = sb.tile([128, CW], BF16, tag="p", bufs=2,
                               name=f"p_{c}_{h}_{tt}")
                nc.scalar.activation(
                    out=p_sb,
                    in_=s_cur,
                    func=mybir.ActivationFunctionType.Exp,
                    bias=maskb[:, tt : tt + 1],
                    scale=1.0,
                )
                if i + 1 < len(tiles):
                    s_cur = emit_qk(r, *tiles[i + 1])
                elif carry_out:
                    carry[r + 1] = emit_qk(r + 1, *tiles[0])
                for half in range(CW // 512):
                    nc.tensor.matmul(
                        out=z_ps[:, ds(half * 512, 512)],
                        lhsT=t["vp"][:, tt, h, :],
                        rhs=p_sb[:, ds(half * 512, 512)],
                        start=(tt == 0),
                        stop=(tt == ST - 1),
                    )
                if tt == ST - 1 and "nonorm" not in ablate:
                    # normalize: 1/denominator broadcast down 64 partitions
                    # on GPSIMD keeps the whole chain off the PE and ACT
                    z_sb = sb.tile([DH + 1, CW], F32, tag="zsb", bufs=2,
                                   name=f"z_sb_{c}_{h}")
                    nc.vector.tensor_copy(out=z_sb, in_=z_ps)
                    recip = sb.tile([1, CW], F32, tag="recip", bufs=2,
                                    name=f"recip_{c}_{h}")
                    nc.vector.reciprocal(recip, z_sb[DH : DH + 1, :])
                    bc_sb = sb.tile([64, CW], F32, tag="bc_sb", bufs=2,
                                    name=f"bc_{c}_{h}")
                    nc.gpsimd.partition_broadcast(bc_sb, recip)
                    row = (h % 2) * 64
                    nc.vector.tensor_mul(
                        out=t["attnT"][h // 2][row : row + 64,
                                               ds(c * CW, CW)],
                        in0=z_sb[0:DH, :],
                        in1=bc_sb,
                    )
            main_q.drain()
            late_q.drain()

        # ---- body: software pipeline over reps ----
        state[0] = alloc_rep(0)
        run_gens([gen_dma_in(state[0]), gen_prep(state[0])])
        run_gens(proj_gens(state[0]))

        for r in range(reps):
            main = []
            if r >= 1 and "nooutproj" not in ablate:
                # previous rep's chunk-1 out-projection
                for st0 in range(8, ST, 2):
                    main.append(gen_outproj_unit(state[r - 1], st0))
            if r + 1 < reps:
                state[r + 1] = alloc_rep(r + 1)
                main.append(gen_dma_in(state[r + 1]))
                main.append(gen_prep(state[r + 1]))
                if "noproj" not in ablate:
                    main.extend(proj_gens(state[r + 1]))
            late = ([] if "nooutproj" in ablate else
                    [gen_outproj_unit(state[r], st0) for st0 in range(0, 8, 2)])
            emit_attention(r, Stepper(main), Stepper(late),
                           carry_out=(r + 1 < reps))

        # tail: last rep's chunk-1 out-projection
        if "nooutproj" not in ablate:
            run_gens([gen_outproj_unit(state[reps - 1], st0)
                      for st0 in range(8, ST, 2)])

    nc.compile()
    return nc


def kernel(
    x, mask, Wq, bq, Wk, bk, Wv, bv, Wo, bo, pos_embed, alpha, beta, **_unused
):
    x = np.asarray(x, dtype=np.float32)
    mask = np.asarray(mask)
    Wq = np.asarray(Wq, dtype=np.float32)
    Wk = np.asarray(Wk, dtype=np.float32)
    Wv = np.asarray(Wv, dtype=np.float32)
    Wo = np.asarray(Wo, dtype=np.float32)
    bq = np.asarray(bq, dtype=np.float32)
    bv = np.asarray(bv, dtype=np.float32)
    bo = np.asarray(bo, dtype=np.float32)
    pe = np.asarray(pos_embed, dtype=np.float32)
    alpha = np.asarray(alpha, dtype=np.float32).reshape(H)

    if "nc" not in _CACHE:
        _CACHE["nc"] = _build()
    nc = _CACHE["nc"]

    scale = np.float32(1.0 / np.sqrt(DH))
    peT_np = np.ascontiguousarray(pe.T)
    maskbias = np.where(mask == 0, np.float32(-1e9), np.float32(0.0)).astype(
        np.float32
    )

    in_maps = []
    for core in range(NCORES):
        b, g = divmod(core, GROUPS)
        osl = slice(g * O, (g + 1) * O)
        heads = list(range(g * HPC, (g + 1) * HPC))
        smalls = np.zeros((128, 2 + ST + HPC), np.float32)
        smalls[:, 0:2] = (bq[osl] * scale).reshape(2, 128).T
        smalls[:, 2 : 2 + ST] = maskbias[b].reshape(ST, 128).T
        smalls[0:64, 2 + ST :] = alpha[heads][None, :]

        def sb_layout(mat_T, kt):
            # [rows, cols] -> [128, kt, cols]: row r = k*128 + p -> [p][k]
            r, cols = mat_T.shape
            return np.ascontiguousarray(
                mat_T.reshape(kt, 128, cols).transpose(1, 0, 2)
            )

        in_maps.append(
            {
                "xT": sb_layout(x[b].T, KD).astype(BF16_NP),
                "wqT": sb_layout((Wq[osl] * scale).T, KD).astype(BF16_NP),
                "wkT": sb_layout(Wk[osl].T, KD).astype(BF16_NP),
                "wvT": sb_layout(Wv[osl].T, KD).astype(BF16_NP),
                "woT": sb_layout(Wo[:, osl].T, 2).astype(BF16_NP),
                "peT": peT_np.astype(BF16_NP),
                "smalls": smalls,
                "out": np.zeros((S, D), BF16_NP),
            }
        )

    _CACHE["in_maps"] = in_maps
    # the axon-tunneled devices intermittently fault (NRT_EXEC_UNIT_
    # UNRECOVERABLE); a retry on a fresh call recovers
    for attempt in range(3):
        try:
            res = run_bass_kernel_spmd(nc, in_maps, core_ids=list(range(NCORES)))
            break
        except Exception:
            if attempt == 2:
                raise

    correction = Wo @ bv + bo  # exact bv/bo contribution (see module docstring)
    out = np.empty((B, S, D), np.float32)
    for b in range(B):
        acc = np.zeros((S, D), np.float64)
        for g in range(GROUPS):
            acc += res.results[b * GROUPS + g]["out"].astype(np.float32)
        out[b] = (acc + correction).astype(np.float32)
    return out
